# revision 20
# baseline (speedup 1.0000x reference)
"""MDCT (conv1d stride-512, kernel-1024, pad-512) as a Bass/Tile kernel on 8 trn2 cores.

Strategy
--------
out[b,k,j] = sum_t F[k,t] * xpad[b, j*512 + t],  x:[16,1,1048576] -> out:[16,512,2049]

* Data-parallel over batch: 2 batches per NeuronCore (8 cores).
* MDCT fold halves the matmul contraction (2N=1024 window -> N=512 DCT-IV):
    frame j window = [A_{j-1}, A_j]  (A_j = x[j*512:(j+1)*512])
    g2'[q] = A[q] + A[511-q]   (q in [0,256))   [= g2 reversed]
    g1 [q] = A[q] - A[511-q]
    out[:,j] = W2' @ g2'(A_j) + W1 @ g1(A_{j-1})
  where the weight matrices come from the filter itself (least-squares over the
  two redundant copies of each coefficient present in F), with the u-reversal
  of g2 absorbed into a host-side weight column permutation.
* Transpose-free: the host delivers x as two bf16 planes xp0[q,j]=A_j[q],
  xp1[q,j]=A_j[511-q] (a pure layout permutation), so the fold is a plain
  partition-aligned add/sub on the DVE and its outputs land directly in
  [contraction, frame] layout as matmul moving operands. No PE transposes,
  no PSUM staging of the rhs.
* bf16 end-to-end (inputs, weights, outputs) with fp32 PSUM accumulation;
  host upcasts the result to fp32.
"""

import numpy as np

N = 512
B = 16
T = 2048
NCORES = 8
BPC = B // NCORES          # batches per core = 2
JCHUNK = 512               # frames per chunk (PSUM bank = 512 fp32)
NCHUNK = T // JCHUNK       # 4 full chunks; frame 2048 handled as tail
NWARM = 2                  # PE warmup matmuls bridging the startup window

_compiled = None


def _build():
    import concourse.mybir as mybir
    from concourse import bacc
    from concourse.tile import TileContext

    f32 = mybir.dt.float32
    bf16 = mybir.dt.bfloat16

    nc = bacc.Bacc("TRN2", target_bir_lowering=False, debug=False)

    # xp[b, c, qc, p, j]: c=0 plane A_j[q], c=1 plane A_j[511-q], q = 128*qc+p
    xp_d = nc.dram_tensor("xp", [BPC, 2, 2, 128, T], bf16, kind="ExternalInput").ap()
    w_d = nc.dram_tensor("wt", [4, 128, N], bf16, kind="ExternalInput").ap()
    o_d = nc.dram_tensor("os", [BPC, N, T + 1], bf16, kind="ExternalOutput").ap()

    with TileContext(nc) as tc:
        with tc.tile_pool(name="wp", bufs=1) as wp, \
             tc.tile_pool(name="xp", bufs=8) as xpool, \
             tc.tile_pool(name="g2p", bufs=6) as g2p, \
             tc.tile_pool(name="g1p", bufs=6) as g1p, \
             tc.tile_pool(name="op", bufs=4) as op, \
             tc.tile_pool(name="ops", bufs=8, space="PSUM") as ops:

            def load_x(b, jc, eng0=None):
                # pair-plane tiles [128 q, 2 c, 513 j] (cols j0-1..j0+511)
                j0 = jc * JCHUNK
                X = []
                for qc in range(2):
                    dma = (eng0 if qc == 0 and eng0 is not None
                           else nc.sync).dma_start
                    x_t = xpool.tile([128, 2, JCHUNK + 1], bf16, tag=f"x{qc}")
                    if jc == 0:
                        nc.vector.memset(x_t[:, :, 0:1], 0.0)
                        dma(
                            out=x_t[:, :, 1:JCHUNK + 1],
                            in_=xp_d[b, :, qc, :, 0:JCHUNK].rearrange(
                                "c p j -> p c j"),
                        )
                    else:
                        dma(
                            out=x_t[:],
                            in_=xp_d[b, :, qc, :, j0 - 1:j0 + JCHUNK].rearrange(
                                "c p j -> p c j"),
                        )
                    X.append(x_t)
                return X

            # warmup: keep the PE continuously busy on zeroed scratch through
            # the DMA/fold startup window, so the p-state ramp completes and
            # every real matmul runs at full clock with no idle-gap resets
            scr = wp.tile([128, JCHUNK], bf16, tag="scr")
            nc.gpsimd.memset(scr[:], 0.0)
            spo = ops.tile([128, JCHUNK], f32, tag="po", name="spo")
            for _ in range(NWARM):
                nc.tensor.matmul(spo[:], scr[:, 0:128], scr[:],
                                 start=True, stop=True)

            def fold(b, jc, X):
                # ---- fold: g2' = p0+p1 (frames j0..), g1 = p0-p1 (shifted)
                first = b == 0 and jc == 0
                w1 = JCHUNK + 1 if jc == NCHUNK - 1 else JCHUNK
                G2, G1 = [None, None], [None, None]
                # first chunk: split folds across DVE+Pool, last-consumed
                # first, so the PE never stalls once it starts
                fold_order = ((0, False), (1, False), (0, True), (1, True)) \
                    if first else ((0, True), (0, False), (1, True), (1, False))
                for qc, is_g2 in fold_order:
                    if is_g2:
                        g2_t = g2p.tile([128, JCHUNK], bf16, tag=f"g2{qc}")
                        eng = nc.vector if not first or qc == 1 else nc.gpsimd
                        eng.tensor_add(
                            g2_t[:],
                            X[qc][:, 0, 1:JCHUNK + 1],
                            X[qc][:, 1, 1:JCHUNK + 1])
                        G2[qc] = g2_t
                    else:
                        g1_t = g1p.tile([128, JCHUNK + 1], bf16, tag=f"g1{qc}")
                        eng = nc.vector if not first or qc == 1 else nc.gpsimd
                        eng.tensor_sub(
                            g1_t[:, 0:w1],
                            X[qc][:, 0, 0:w1], X[qc][:, 1, 0:w1])
                        G1[qc] = g1_t
                return G2, G1

            # first input tiles go ahead of the weights on the DMA queue so
            # the fold (the critical path to the first matmul) starts ASAP;
            # weights then arrive ordered to pipeline against the
            # uc-(2,3,1,0) matmul order of the first chunk
            X0 = load_x(0, 0)
            W = [None] * 4
            for uc in (2, 3, 1, 0):
                w_t = wp.tile([128, N], bf16, tag=f"w{uc}", name=f"w{uc}")
                nc.sync.dma_start(out=w_t[:], in_=w_d[uc])
                W[uc] = w_t

            items = [(b, jc) for b in range(BPC) for jc in range(NCHUNK)]
            folded = {(0, 0): fold(0, 0, X0)}
            for i, (b, jc) in enumerate(items):
                j0 = jc * JCHUNK
                first = b == 0 and jc == 0
                G2, G1 = folded.pop((b, jc))
                # prefetch the next chunk's loads+folds ahead of this chunk's
                # matmuls/copies so the DVE fold stream stays ahead of the PE
                if i + 1 < len(items):
                    nb, njc = items[i + 1]
                    folded[(nb, njc)] = fold(nb, njc, load_x(nb, njc))

                if True:
                    if jc == NCHUNK - 1:
                        # tail frame j=2048 (= W2@g1lo + W3@g1hi at col 512),
                        # hoisted before the chunk matmuls so its copy/DMA
                        # drain behind the chunk's PE work
                        otail = op.tile([128, 4], bf16, tag="otail")
                        for kc in range(4):
                            pt = ops.tile([128, JCHUNK], f32, tag="po",
                                          name="pt")
                            ks = slice(128 * kc, 128 * (kc + 1))
                            nc.tensor.matmul(pt[:, 0:1], W[2][:, ks],
                                             G1[0][:, JCHUNK:JCHUNK + 1],
                                             start=True, stop=False)
                            nc.tensor.matmul(pt[:, 0:1], W[3][:, ks],
                                             G1[1][:, JCHUNK:JCHUNK + 1],
                                             start=False, stop=True)
                            nc.scalar.copy(out=otail[:, kc:kc + 1],
                                           in_=pt[:, 0:1])
                        nc.sync.dma_start(
                            out=o_d[b, :, T:T + 1].rearrange(
                                "(c p) o -> p (c o)", p=128),
                            in_=otail[:],
                        )

                    # ---- matmuls: po = W0@g2'lo + W1@g2'hi + W2@g1lo + W3@g1hi
                    last = b == BPC - 1 and jc == NCHUNK - 1
                    ot = None if last else op.tile([128, 4, JCHUNK], bf16,
                                                   tag="o")
                    RHS = [G2[0][:], G2[1][:], G1[0][:, 0:JCHUNK],
                           G1[1][:, 0:JCHUNK]]
                    PO = [ops.tile([128, JCHUNK], f32, tag="po", name=f"po{i}")
                          for i in range(4)]
                    if first:
                        # uc order (2,3,1,0), kc-inner: each weight tile and
                        # fold output is consumed right as it lands, with
                        # zero PE stalls
                        for uc in (2, 3, 1, 0):
                            for kc in range(4):
                                ks = slice(128 * kc, 128 * (kc + 1))
                                nc.tensor.matmul(PO[kc][:], W[uc][:, ks],
                                                 RHS[uc], start=(uc == 2),
                                                 stop=(uc == 0))
                    else:
                        for kc in range(4):
                            ks = slice(128 * kc, 128 * (kc + 1))
                            for uc in range(4):
                                nc.tensor.matmul(PO[kc][:], W[uc][:, ks],
                                                 RHS[uc], start=(uc == 0),
                                                 stop=(uc == 3))
                    for kc in range(4):
                        cp = nc.scalar.copy if kc % 2 == 0 else nc.vector.tensor_copy
                        if last:
                            # final chunk: per-kc staging + DMA so the drain
                            # pipelines instead of waiting for all 4 copies;
                            # the very last copy is split across both
                            # PSUM-capable engines to launch the final DMA
                            # sooner
                            ok = op.tile([128, JCHUNK], bf16, tag="ok")
                            cp(out=ok[:], in_=PO[kc][:])
                            nc.sync.dma_start(
                                out=o_d[b, 128 * kc:128 * (kc + 1),
                                        j0:j0 + JCHUNK],
                                in_=ok[:],
                            )
                        else:
                            cp(out=ot[:, kc], in_=PO[kc][:])
                    if not last:
                        nc.gpsimd.dma_start(
                            out=o_d[b, :, j0:j0 + JCHUNK].rearrange(
                                "(c p) j -> p c j", p=128),
                            in_=ot[:],
                        )

    nc.compile()
    return nc


def _weights(mdct_filter: np.ndarray) -> np.ndarray:
    """Extract DCT-IV weight tiles W[4,128,512] from the 1024-tap filter.

    Each coefficient appears twice in F (up to sign); average the two copies
    (least squares) to minimize the fold residual. Column order matches the
    on-device g2'/g1 fold layout (g2 reversed into g2').
    """
    F = mdct_filter.reshape(N, 2 * N).astype(np.float64)
    sideA = np.concatenate([-F[:, 768:1024], F[:, 0:256]], axis=1)
    sideB = -F[:, 767:255:-1]
    Cp = 0.5 * (sideA + sideB)  # [k, u]
    W = np.empty((4, 128, N), dtype=np.float64)
    W[0] = -Cp[:, 255:127:-1].T   # g2' lo: row q ↔ u = 255-q
    W[1] = -Cp[:, 127::-1].T      # g2' hi: row q ↔ u = 127-q
    W[2] = Cp[:, 256:384].T       # g1 lo
    W[3] = Cp[:, 384:512].T       # g1 hi
    return W


def kernel(x: np.ndarray, mdct_filter: np.ndarray, _trace=False) -> np.ndarray:
    global _compiled
    import ml_dtypes
    from concourse.bass_utils import run_bass_kernel_spmd

    bf16 = ml_dtypes.bfloat16
    if _compiled is None:
        _compiled = _build()
    nc = _compiled

    xr = np.ascontiguousarray(np.asarray(x, dtype=np.float32)).reshape(B, T, N)
    xp0 = xr[:, :, 0:256].transpose(0, 2, 1)             # [B, 256, T] = A_j[q]
    xp1 = xr[:, :, 256:512][:, :, ::-1].transpose(0, 2, 1)  # A_j[511-q]
    xp = np.stack([xp0, xp1], axis=1).astype(bf16).reshape(B, 2, 2, 128, T)
    wt = _weights(np.asarray(mdct_filter, dtype=np.float32)).astype(bf16)

    in_maps = [
        {"xp": xp[c * BPC:(c + 1) * BPC], "wt": wt}
        for c in range(NCORES)
    ]
    res = run_bass_kernel_spmd(nc, in_maps, core_ids=list(range(NCORES)),
                               trace=_trace)
    out = np.empty((B, N, T + 1), dtype=np.float32)
    for c in range(NCORES):
        out[c * BPC:(c + 1) * BPC] = np.asarray(
            res.results[c]["os"]).astype(np.float32)
    if _trace:
        kernel._last_results = res
    return out


# revision 23
# speedup vs baseline: 1.0064x; 1.0064x over previous
"""MDCT (conv1d stride-512, kernel-1024, pad-512) as a Bass/Tile kernel on 8 trn2 cores.

Strategy
--------
out[b,k,j] = sum_t F[k,t] * xpad[b, j*512 + t],  x:[16,1,1048576] -> out:[16,512,2049]

* Data-parallel over batch: 2 batches per NeuronCore (8 cores).
* MDCT fold halves the matmul contraction (2N=1024 window -> N=512 DCT-IV):
    frame j window = [A_{j-1}, A_j]  (A_j = x[j*512:(j+1)*512])
    g2'[q] = A[q] + A[511-q]   (q in [0,256))   [= g2 reversed]
    g1 [q] = A[q] - A[511-q]
    out[:,j] = W2' @ g2'(A_j) + W1 @ g1(A_{j-1})
  where the weight matrices come from the filter itself (least-squares over the
  two redundant copies of each coefficient present in F), with the u-reversal
  of g2 absorbed into a host-side weight column permutation.
* Transpose-free: the host delivers x as two bf16 planes xp0[q,j]=A_j[q],
  xp1[q,j]=A_j[511-q] (a pure layout permutation), so the fold is a plain
  partition-aligned add/sub on the DVE and its outputs land directly in
  [contraction, frame] layout as matmul moving operands. No PE transposes,
  no PSUM staging of the rhs.
* bf16 end-to-end (inputs, weights, outputs) with fp32 PSUM accumulation;
  host upcasts the result to fp32.
"""

import numpy as np

N = 512
B = 16
T = 2048
NCORES = 8
BPC = B // NCORES          # batches per core = 2
JCHUNK = 512               # frames per chunk (PSUM bank = 512 fp32)
NCHUNK = T // JCHUNK       # 4 full chunks; frame 2048 handled as tail
NWARM = 2                  # PE warmup matmuls bridging the startup window

_compiled = None


def _build():
    import concourse.mybir as mybir
    from concourse import bacc
    from concourse.tile import TileContext

    f32 = mybir.dt.float32
    bf16 = mybir.dt.bfloat16

    nc = bacc.Bacc("TRN2", target_bir_lowering=False, debug=False)

    # xp[b, c, qc, p, j]: c=0 plane A_j[q], c=1 plane A_j[511-q], q = 128*qc+p
    xp_d = nc.dram_tensor("xp", [BPC, 2, 2, 128, T], bf16, kind="ExternalInput").ap()
    w_d = nc.dram_tensor("wt", [4, 128, N], bf16, kind="ExternalInput").ap()
    o_d = nc.dram_tensor("os", [BPC, N, T + 1], bf16, kind="ExternalOutput").ap()

    with TileContext(nc) as tc:
        with tc.tile_pool(name="wp", bufs=1) as wp, \
             tc.tile_pool(name="xp", bufs=8) as xpool, \
             tc.tile_pool(name="g2p", bufs=6) as g2p, \
             tc.tile_pool(name="g1p", bufs=6) as g1p, \
             tc.tile_pool(name="op", bufs=4) as op, \
             tc.tile_pool(name="ops", bufs=8, space="PSUM") as ops:

            def load_x(b, jc, eng0=None):
                # pair-plane tiles [128 q, 2 c, 513 j] (cols j0-1..j0+511)
                j0 = jc * JCHUNK
                X = []
                for qc in range(2):
                    dma = (eng0 if qc == 0 and eng0 is not None
                           else nc.sync).dma_start
                    x_t = xpool.tile([128, 2, JCHUNK + 1], bf16, tag=f"x{qc}")
                    if jc == 0:
                        nc.vector.memset(x_t[:, :, 0:1], 0.0)
                        dma(
                            out=x_t[:, :, 1:JCHUNK + 1],
                            in_=xp_d[b, :, qc, :, 0:JCHUNK].rearrange(
                                "c p j -> p c j"),
                        )
                    else:
                        dma(
                            out=x_t[:],
                            in_=xp_d[b, :, qc, :, j0 - 1:j0 + JCHUNK].rearrange(
                                "c p j -> p c j"),
                        )
                    X.append(x_t)
                return X

            # warmup: keep the PE continuously busy on zeroed scratch through
            # the DMA/fold startup window, so the p-state ramp completes and
            # every real matmul runs at full clock with no idle-gap resets
            scr = wp.tile([128, JCHUNK], bf16, tag="scr")
            nc.gpsimd.memset(scr[:], 0.0)
            spo = ops.tile([128, JCHUNK], f32, tag="po", name="spo")
            for _ in range(NWARM):
                nc.tensor.matmul(spo[:], scr[:, 0:128], scr[:],
                                 start=True, stop=True)

            def fold(b, jc, X):
                # ---- fold: g2' = p0+p1 (frames j0..), g1 = p0-p1 (shifted)
                first = b == 0 and jc == 0
                w1 = JCHUNK + 1 if jc == NCHUNK - 1 else JCHUNK
                G2, G1 = [None, None], [None, None]
                # first chunk: split folds across DVE+Pool, last-consumed
                # first, so the PE never stalls once it starts
                fold_order = ((0, False), (1, False), (0, True), (1, True)) \
                    if first else ((0, True), (0, False), (1, True), (1, False))
                for qc, is_g2 in fold_order:
                    eng = nc.vector if not first or qc == 0 else nc.gpsimd
                    if is_g2:
                        g2_t = g2p.tile([128, JCHUNK], bf16, tag=f"g2{qc}")
                        eng.tensor_add(
                            g2_t[:],
                            X[qc][:, 0, 1:JCHUNK + 1],
                            X[qc][:, 1, 1:JCHUNK + 1])
                        G2[qc] = g2_t
                    else:
                        g1_t = g1p.tile([128, JCHUNK + 1], bf16, tag=f"g1{qc}")
                        eng.tensor_sub(
                            g1_t[:, 0:w1],
                            X[qc][:, 0, 0:w1], X[qc][:, 1, 0:w1])
                        G1[qc] = g1_t
                return G2, G1

            # first input tiles go ahead of the weights on the DMA queue so
            # the fold (the critical path to the first matmul) starts ASAP;
            # weights then arrive ordered to pipeline against the
            # uc-(2,3,1,0) matmul order of the first chunk
            X0 = load_x(0, 0)
            W = [None] * 4
            for uc in (2, 3, 1, 0):
                w_t = wp.tile([128, N], bf16, tag=f"w{uc}", name=f"w{uc}")
                nc.sync.dma_start(out=w_t[:], in_=w_d[uc])
                W[uc] = w_t

            items = [(b, jc) for b in range(BPC) for jc in range(NCHUNK)]
            folded = {(0, 0): fold(0, 0, X0)}
            for i, (b, jc) in enumerate(items):
                j0 = jc * JCHUNK
                first = b == 0 and jc == 0
                G2, G1 = folded.pop((b, jc))
                # prefetch the next chunk's loads+folds ahead of this chunk's
                # matmuls/copies so the DVE fold stream stays ahead of the PE
                if i + 1 < len(items):
                    nb, njc = items[i + 1]
                    folded[(nb, njc)] = fold(nb, njc, load_x(nb, njc))

                if True:
                    if jc == NCHUNK - 1:
                        # tail frame j=2048 (= W2@g1lo + W3@g1hi at col 512),
                        # hoisted before the chunk matmuls so its copy/DMA
                        # drain behind the chunk's PE work
                        otail = op.tile([128, 4], bf16, tag="otail")
                        for kc in range(4):
                            pt = ops.tile([128, JCHUNK], f32, tag="po",
                                          name="pt")
                            ks = slice(128 * kc, 128 * (kc + 1))
                            nc.tensor.matmul(pt[:, 0:1], W[2][:, ks],
                                             G1[0][:, JCHUNK:JCHUNK + 1],
                                             start=True, stop=False)
                            nc.tensor.matmul(pt[:, 0:1], W[3][:, ks],
                                             G1[1][:, JCHUNK:JCHUNK + 1],
                                             start=False, stop=True)
                            nc.scalar.copy(out=otail[:, kc:kc + 1],
                                           in_=pt[:, 0:1])
                        nc.sync.dma_start(
                            out=o_d[b, :, T:T + 1].rearrange(
                                "(c p) o -> p (c o)", p=128),
                            in_=otail[:],
                        )

                    # ---- matmuls: po = W0@g2'lo + W1@g2'hi + W2@g1lo + W3@g1hi
                    last = b == BPC - 1 and jc == NCHUNK - 1
                    ot = None if last else op.tile([128, 4, JCHUNK], bf16,
                                                   tag="o")
                    RHS = [G2[0][:], G2[1][:], G1[0][:, 0:JCHUNK],
                           G1[1][:, 0:JCHUNK]]
                    PO = [ops.tile([128, JCHUNK], f32, tag="po", name=f"po{i}")
                          for i in range(4)]
                    if first:
                        # uc order (2,3,1,0), kc-inner: each weight tile and
                        # fold output is consumed right as it lands, with
                        # zero PE stalls
                        for uc in (2, 3, 1, 0):
                            for kc in range(4):
                                ks = slice(128 * kc, 128 * (kc + 1))
                                nc.tensor.matmul(PO[kc][:], W[uc][:, ks],
                                                 RHS[uc], start=(uc == 2),
                                                 stop=(uc == 0))
                    else:
                        for kc in range(4):
                            ks = slice(128 * kc, 128 * (kc + 1))
                            for uc in range(4):
                                nc.tensor.matmul(PO[kc][:], W[uc][:, ks],
                                                 RHS[uc], start=(uc == 0),
                                                 stop=(uc == 3))
                    for kc in range(4):
                        cp = nc.scalar.copy if kc % 2 == 0 else nc.vector.tensor_copy
                        if last:
                            # final chunk: per-kc staging + DMA so the drain
                            # pipelines instead of waiting for all 4 copies;
                            # the very last copy is split across both
                            # PSUM-capable engines to launch the final DMA
                            # sooner
                            ok = op.tile([128, JCHUNK], bf16, tag="ok")
                            cp(out=ok[:], in_=PO[kc][:])
                            nc.sync.dma_start(
                                out=o_d[b, 128 * kc:128 * (kc + 1),
                                        j0:j0 + JCHUNK],
                                in_=ok[:],
                            )
                        else:
                            cp(out=ot[:, kc], in_=PO[kc][:])
                    if not last:
                        nc.gpsimd.dma_start(
                            out=o_d[b, :, j0:j0 + JCHUNK].rearrange(
                                "(c p) j -> p c j", p=128),
                            in_=ot[:],
                        )

    nc.compile()
    return nc


def _weights(mdct_filter: np.ndarray) -> np.ndarray:
    """Extract DCT-IV weight tiles W[4,128,512] from the 1024-tap filter.

    Each coefficient appears twice in F (up to sign); average the two copies
    (least squares) to minimize the fold residual. Column order matches the
    on-device g2'/g1 fold layout (g2 reversed into g2').
    """
    F = mdct_filter.reshape(N, 2 * N).astype(np.float64)
    sideA = np.concatenate([-F[:, 768:1024], F[:, 0:256]], axis=1)
    sideB = -F[:, 767:255:-1]
    Cp = 0.5 * (sideA + sideB)  # [k, u]
    W = np.empty((4, 128, N), dtype=np.float64)
    W[0] = -Cp[:, 255:127:-1].T   # g2' lo: row q ↔ u = 255-q
    W[1] = -Cp[:, 127::-1].T      # g2' hi: row q ↔ u = 127-q
    W[2] = Cp[:, 256:384].T       # g1 lo
    W[3] = Cp[:, 384:512].T       # g1 hi
    return W


def kernel(x: np.ndarray, mdct_filter: np.ndarray, _trace=False) -> np.ndarray:
    global _compiled
    import ml_dtypes
    from concourse.bass_utils import run_bass_kernel_spmd

    bf16 = ml_dtypes.bfloat16
    if _compiled is None:
        _compiled = _build()
    nc = _compiled

    xr = np.ascontiguousarray(np.asarray(x, dtype=np.float32)).reshape(B, T, N)
    xp0 = xr[:, :, 0:256].transpose(0, 2, 1)             # [B, 256, T] = A_j[q]
    xp1 = xr[:, :, 256:512][:, :, ::-1].transpose(0, 2, 1)  # A_j[511-q]
    xp = np.stack([xp0, xp1], axis=1).astype(bf16).reshape(B, 2, 2, 128, T)
    wt = _weights(np.asarray(mdct_filter, dtype=np.float32)).astype(bf16)

    in_maps = [
        {"xp": xp[c * BPC:(c + 1) * BPC], "wt": wt}
        for c in range(NCORES)
    ]
    res = run_bass_kernel_spmd(nc, in_maps, core_ids=list(range(NCORES)),
                               trace=_trace)
    out = np.empty((B, N, T + 1), dtype=np.float32)
    for c in range(NCORES):
        out[c * BPC:(c + 1) * BPC] = np.asarray(
            res.results[c]["os"]).astype(np.float32)
    if _trace:
        kernel._last_results = res
    return out
